# revision 26
# baseline (speedup 1.0000x reference)
"""Multi-head attention Trainium2 kernel (B=4, S=2048, D=1024, H=16, causal).

Sharding: 8 cores = 4 batches x 2 head-groups (8 heads each, tensor-parallel
over the QKV/out projection weights along the head dimension).

Single software-pipelined pass per core (no serial phases): stage ts in 0..3
computes q-block ts of the causal attention; the projections for s-block
ts+1 and the output projection of earlier q-blocks are interleaved into the
(ACT-paced) attention loop as PE filler so the tensor engine never waits on
the exp chain.

  - host supplies transposed activations xT [D, S] and weights in bf16
    (halves DMA; matmuls run at full PE rate either way, accumulation stays
    fp32 in PSUM).  x/w loads are merged into few descriptor-rich DMAs (SP
    DGE config costs 565ns per dma_start); stage-0 loads are laddered in
    d-chunks of (1,1,2,4) so the first matmul starts ~3us in.
  - small loads (biases, masks) go through the GpSimd SWDGE queue.
  - projections produce qhT/khT head-major [o, s] (bias folded into an ACT
    Identity+bias op straight out of PSUM) and vh sequence-major
    [s, (h, dk+1)] with a ones column for the softmax denominator.
  - stage-0 projections run contraction-outer across four PSUM banks so the
    PE streams behind the arriving x DMA chunks.
  - scoresT[k, q] per head pair in one 2-bank PSUM tile; exp on ACT with the
    1/sqrt(dk) scale folded in writes bf16 e01; causal strip masked by a
    bf16 DVE multiply.
  - ctx accumulation per head into [dk+1, q] PSUM; denominator in row 64.
    Normalize (emitted after the next pair's first scores so the exp chain
    never waits): DVE reciprocal_approx_fast on the denominator rows + PSUM
    evacuation copies split across ACT/DVE (frees the accumulator banks
    early), then GpSimd partition_broadcast + GpSimd multiplies.
  - output projection consumes the d'-major bf16 ctxT; the last q-block's
    projection pre-runs its first three weight tiles across six PSUM banks
    while the final normalize drains.  Per-core bf16 partials are summed
    pairwise (+ bo) in fp32 on the host.
"""

import numpy as np
import ml_dtypes

import concourse.bacc as bacc
import concourse.mybir as mybir
import concourse.tile as tile
from concourse.bass_utils import run_bass_kernel_spmd

B, S, D, H = 4, 2048, 1024, 16
DK = D // H          # 64
N_CORES = 8
O = 512              # head dims per core (8 heads x 64)
HPC = 8              # heads per core
SB = 512             # s-block (= stage granularity = q-block)
QB = 512
KT = 128             # k tile
F32 = mybir.dt.float32
BF16 = mybir.dt.bfloat16
AF = mybir.ActivationFunctionType

QCH = [(0, 1), (1, 2), (2, 4), (4, 8)]   # stage-0 d-chunk ladder



_CACHE = {}


def _build(s=S):
    nc = bacc.Bacc("TRN2", target_bir_lowering=False, debug=False,
                   num_devices=N_CORES)
    n_st = s // SB            # pipeline stages / q-blocks / s-blocks
    n_sc = s // 128           # s chunks of 128

    xqT = nc.declare_dram_parameter("xqT", [D, s], BF16, isOutput=False)
    xkT = nc.declare_dram_parameter("xkT", [D, s], BF16, isOutput=False)
    xvT = nc.declare_dram_parameter("xvT", [D, s], BF16, isOutput=False)
    # stage-0 packed loads: row d = [wT[d, :] | xT[d, 0:SB]] so one DMA per
    # d-chunk delivers both the weight tile and the first x block
    qpk = nc.declare_dram_parameter("qpk", [D, O + SB], BF16, isOutput=False)
    kpk = nc.declare_dram_parameter("kpk", [D, O + SB], BF16, isOutput=False)
    vpk = nc.declare_dram_parameter("vpk", [D, O + SB], BF16, isOutput=False)
    bqd = nc.declare_dram_parameter("bq", [O], F32, isOutput=False)
    bkd = nc.declare_dram_parameter("bk", [O], F32, isOutput=False)
    bvb = nc.declare_dram_parameter("bv_bc", [128, O], BF16, isOutput=False)
    wod = nc.declare_dram_parameter("woT", [O, D], BF16, isOutput=False)
    maskd = nc.declare_dram_parameter("masks", [KT, KT], BF16, isOutput=False)
    onesd = nc.declare_dram_parameter("ones8", [128, HPC], BF16, isOutput=False)
    outd = nc.declare_dram_parameter("out", [s, D], BF16, isOutput=True)

    scale = float(DK) ** -0.5

    xq_r = xqT.ap().rearrange("(a p) s -> p a s", p=128)
    xk_r = xkT.ap().rearrange("(a p) s -> p a s", p=128)
    xv_r = xvT.ap().rearrange("(a p) s -> p a s", p=128)
    qpk_r = qpk.ap().rearrange("(a p) o -> p a o", p=128)
    kpk_r = kpk.ap().rearrange("(a p) o -> p a o", p=128)
    vpk_r = vpk.ap().rearrange("(a p) o -> p a o", p=128)
    wo_r = wod.ap().rearrange("(a p) d -> p a d", p=128)

    with tile.TileContext(nc) as tc:
        with (
            tc.tile_pool(name="res", bufs=1) as res,
            tc.tile_pool(name="xpool", bufs=2) as xpool,
            tc.tile_pool(name="epool", bufs=3) as epool,
            tc.tile_pool(name="npool", bufs=1) as npool,
            tc.tile_pool(name="outpool", bufs=4) as outpool,
        ):
            psum = tc.alloc_tile_pool(name="psum", bufs=1, space="PSUM")

            # ---- persistent tiles ----
            qhT = [[res.tile([128, SB], BF16, tag=f"qhT{ts}_{j}",
                             name=f"qhT{ts}_{j}") for j in range(4)]
                   for ts in range(n_st)]
            khT = [[res.tile([128, SB], BF16, tag=f"khT{ts}_{j}",
                             name=f"khT{ts}_{j}") for j in range(4)]
                   for ts in range(n_st)]
            vh = [res.tile([128, HPC, DK + 1], BF16, tag=f"vh{i}",
                           name=f"vh{i}") for i in range(n_sc)]
            ctxT = [[res.tile([128, SB], BF16, tag=f"ctxT{ts}_{j}",
                              name=f"ctxT{ts}_{j}") for j in range(4)]
                    for ts in range(n_st)]
            wq_c = [res.tile([128, e - b, O + SB], BF16, tag=f"wqc{i}",
                             name=f"wqc{i}") for i, (b, e) in enumerate(QCH)]
            wk_m = res.tile([128, 8, O + SB], BF16, tag="wk_m", name="wk_m")
            wv_m = res.tile([128, 8, O + SB], BF16, tag="wv_m", name="wv_m")
            wo_m = res.tile([128, 4, D], BF16, tag="wo_m", name="wo_m")
            bq_t = res.tile([128, O // 128], F32, tag="bq_t", name="bq_t")
            bk_t = res.tile([128, O // 128], F32, tag="bk_t", name="bk_t")
            bv_t = res.tile([128, O], BF16, tag="bv_t", name="bv_t")
            masks = res.tile([128, KT], BF16, tag="masks", name="masks")

            def wq_sl(d, csl):
                for i, (b, e) in enumerate(QCH):
                    if b <= d < e:
                        return wq_c[i][:, d - b, csl]
                raise AssertionError

            # ---- small loads via the GpSimd SWDGE queue (25ns config);
            # the first stage-0 chunk jumps this idle queue to cut startup ----
            small_eng = nc.gpsimd
            small_eng.dma_start(wq_c[0][:], qpk_r[:, 0:1, :])
            small_eng.dma_start(
                bq_t[:], bqd.ap().rearrange("(m p) -> p m", p=128))
            small_eng.dma_start(
                bk_t[:], bkd.ap().rearrange("(m p) -> p m", p=128))
            small_eng.dma_start(bv_t[:], bvb[:, :])
            small_eng.dma_start(masks[:], maskd[:, :])
            for i in range(n_sc):
                nc.vector.memset(vh[i][:, :, DK], 1.0)

            # ---- bulk loads on SP, laddered for stage-0 streaming ----
            def xq0_sl(d):
                for i, (b, e) in enumerate(QCH):
                    if b <= d < e:
                        return wq_c[i][:, d - b, O:O + SB]
                raise AssertionError

            for i, (b, e) in enumerate(QCH):
                if i > 0:
                    nc.sync.dma_start(wq_c[i][:], qpk_r[:, b:e, :])
                if i == 2:
                    nc.sync.dma_start(wk_m[:, 0:4, :], kpk_r[:, 0:4, :])
            nc.sync.dma_start(wk_m[:, 4:8, :], kpk_r[:, 4:8, :])
            nc.sync.dma_start(wv_m[:], vpk_r[:, :, :])

            xq_b = [None] * n_st
            xk_b = [None] * n_st
            xv_b = [None] * n_st

            def stage_x_dma(ts):
                ssl = slice(ts * SB, (ts + 1) * SB)
                xq_b[ts] = xpool.tile([128, 8, SB], BF16, tag="xqm",
                                      name=f"xq{ts}")
                nc.sync.dma_start(xq_b[ts][:], xq_r[:, :, ssl])
                xk_b[ts] = xpool.tile([128, 8, SB], BF16, tag="xkm",
                                      name=f"xk{ts}")
                nc.sync.dma_start(xk_b[ts][:], xk_r[:, :, ssl])
                xv_b[ts] = xpool.tile([128, 8, SB], BF16, tag="xvm",
                                      name=f"xv{ts}")
                nc.sync.dma_start(xv_b[ts][:], xv_r[:, :, ssl])

            if n_st > 1:
                stage_x_dma(1)
            nc.sync.dma_start(wo_m[:], wo_r[:, :, :])

            # ---- stage-0 projections, contraction-outer, with the q/k/v
            # phases striped across different PSUM banks so no phase waits
            # on the previous phase's consumers ----
            def proj_stage0():
                t4 = ["f0", "f1", "sc4", "c0"]
                t4k = ["c1", "f0", "f1", "sc4"]
                t4v = ["c0", "c1", "f0", "f1"]
                psq = [psum.tile([128, SB], F32, tag=t4[m], name=f"p0q{m}")
                       for m in range(4)]
                for d in range(8):
                    for m in range(4):
                        nc.tensor.matmul(
                            psq[m][:], wq_sl(d, slice(m * 128, (m + 1) * 128)),
                            xq0_sl(d), start=(d == 0), stop=(d == 7))
                for m in range(4):
                    nc.scalar.activation(qhT[0][m][:], psq[m][:], AF.Identity,
                                         bias=bq_t[:, m:m + 1], scale=1.0)
                psk = [psum.tile([128, SB], F32, tag=t4k[m], name=f"p0k{m}")
                       for m in range(4)]
                for d in range(8):
                    for m in range(4):
                        nc.tensor.matmul(
                            psk[m][:], wk_m[:, d, m * 128:(m + 1) * 128],
                            wk_m[:, d, O:O + SB], start=(d == 0), stop=(d == 7))
                for m in range(4):
                    nc.scalar.activation(khT[0][m][:], psk[m][:], AF.Identity,
                                         bias=bk_t[:, m:m + 1], scale=1.0)
                psv = [psum.tile([128, O], F32, tag=t4v[sc], name=f"p0v{sc}")
                       for sc in range(4)]
                for d in range(8):
                    for sc in range(4):
                        nc.tensor.matmul(
                            psv[sc][:],
                            wv_m[:, d, O + sc * 128:O + (sc + 1) * 128],
                            wv_m[:, d, 0:O], start=(d == 0), stop=(d == 7))
                for sc in range(4):
                    nc.vector.tensor_tensor(
                        vh[sc][:, :, 0:DK],
                        psv[sc][:].rearrange("p (h e) -> p h e", e=DK),
                        bv_t[:].rearrange("p (h e) -> p h e", e=DK),
                        op=mybir.AluOpType.add)

            # ---- filler units (run interleaved inside the attention) ----
            fctr = [0]

            def proj_q_unit(ts, m):
                ps = psum.tile([128, SB], F32, tag=f"f{fctr[0] % 2}",
                               name=f"psq{ts}_{m}")
                fctr[0] += 1
                for d in range(8):
                    nc.tensor.matmul(
                        ps[:], wq_sl(d, slice(m * 128, (m + 1) * 128)),
                        xq_b[ts][:, d, :], start=(d == 0), stop=(d == 7))
                nc.scalar.activation(qhT[ts][m][:], ps[:], AF.Identity,
                                     bias=bq_t[:, m:m + 1], scale=1.0)

            def proj_k_unit(ts, m):
                ps = psum.tile([128, SB], F32, tag=f"f{fctr[0] % 2}",
                               name=f"psk{ts}_{m}")
                fctr[0] += 1
                for d in range(8):
                    nc.tensor.matmul(
                        ps[:], wk_m[:, d, m * 128:(m + 1) * 128],
                        xk_b[ts][:, d, :], start=(d == 0), stop=(d == 7))
                nc.scalar.activation(khT[ts][m][:], ps[:], AF.Identity,
                                     bias=bk_t[:, m:m + 1], scale=1.0)

            def proj_v_unit(ts, sc):
                si = ts * (SB // 128) + sc
                ps = psum.tile([128, O], F32, tag=f"f{fctr[0] % 2}",
                               name=f"psv{ts}_{sc}")
                fctr[0] += 1
                for d in range(8):
                    nc.tensor.matmul(
                        ps[:], xv_b[ts][:, d, sc * 128:(sc + 1) * 128],
                        wv_m[:, d, 0:O], start=(d == 0), stop=(d == 7))
                nc.vector.tensor_tensor(
                    vh[si][:, :, 0:DK],
                    ps[:].rearrange("p (h e) -> p h e", e=DK),
                    bv_t[:].rearrange("p (h e) -> p h e", e=DK),
                    op=mybir.AluOpType.add)

            def outproj_unit(qb, sc):
                ot = outpool.tile([128, D], BF16, tag="out_t", name="ot")
                for oc in range(2):
                    osl = slice(oc * 512, (oc + 1) * 512)
                    ps = psum.tile([128, 512], F32, tag=f"f{fctr[0] % 2}",
                                   name=f"pso{qb}_{sc}_{oc}")
                    fctr[0] += 1
                    for jw in range(4):
                        nc.tensor.matmul(
                            ps[:], ctxT[qb][jw][:, sc * 128:(sc + 1) * 128],
                            wo_m[:, jw, osl],
                            start=(jw == 0), stop=(jw == 3))
                    nc.vector.tensor_copy(ot[:, osl], ps[:])
                sg = qb * (SB // 128) + sc
                nc.sync.dma_start(outd[sg * 128:(sg + 1) * 128, :], ot[:])

            def outproj_tail(qb, pendn):
                """Final q-block\'s projection: pre-run the first three
                weight tiles of three (sc, oc) groups on banks the final
                normalize does not read, emit the deferred normalize, then
                finish."""
                tpre = ["f0", "f1", "sc4"]
                trest = ["c0", "c1", "f0", "f1", "sc4"]
                groups = [(sc, oc) for sc in range(4) for oc in range(2)]
                ots = [outpool.tile([128, D], BF16, tag="out_t",
                                    name=f"ott{sc}") for sc in range(4)]

                def tail_copy(sc, oc, ps):
                    osl = slice(oc * 512, (oc + 1) * 512)
                    if oc == 0:
                        nc.scalar.activation(ots[sc][:, osl], ps[:], AF.Copy,
                                             bias=0.0, scale=1.0)
                    else:
                        nc.vector.tensor_copy(ots[sc][:, osl], ps[:])

                def tail_dma(sc, osl=slice(0, D)):
                    sg = qb * (SB // 128) + sc
                    eng = nc.scalar if sc % 2 == 0 else nc.sync
                    eng.dma_start(outd[sg * 128:(sg + 1) * 128, osl],
                                  ots[sc][:, osl])

                pss = {}
                for gi, (sc, oc) in enumerate(groups[:3]):
                    osl = slice(oc * 512, (oc + 1) * 512)
                    ps = psum.tile([128, 512], F32, tag=tpre[gi],
                                   name=f"pst{sc}_{oc}")
                    pss[(sc, oc)] = ps
                    for jw in range(3):
                        nc.tensor.matmul(
                            ps[:], ctxT[qb][jw][:, sc * 128:(sc + 1) * 128],
                            wo_m[:, jw, osl],
                            start=(jw == 0), stop=False)
                pendn()
                for sc, oc in groups[:3]:
                    osl = slice(oc * 512, (oc + 1) * 512)
                    ps = pss[(sc, oc)]
                    nc.tensor.matmul(
                        ps[:], ctxT[qb][3][:, sc * 128:(sc + 1) * 128],
                        wo_m[:, 3, osl], start=False, stop=True)
                    tail_copy(sc, oc, ps)
                    if oc == 1:
                        tail_dma(sc)
                for gi, (sc, oc) in enumerate(groups[3:]):
                    osl = slice(oc * 512, (oc + 1) * 512)
                    ps = psum.tile([128, 512], F32, tag=trest[gi],
                                   name=f"pst2_{sc}_{oc}")
                    for jw in range(4):
                        nc.tensor.matmul(
                            ps[:], ctxT[qb][jw][:, sc * 128:(sc + 1) * 128],
                            wo_m[:, jw, osl],
                            start=(jw == 0), stop=(jw == 3))
                    tail_copy(sc, oc, ps)
                    if sc < 3 and oc == 1:
                        tail_dma(sc)
                    elif sc == 3:
                        tail_dma(sc, osl)

            def make_filler(ts):
                us = []
                if ts + 1 < n_st:
                    for m in range(4):
                        us.append(lambda ts=ts, m=m: proj_q_unit(ts + 1, m))
                if ts == 1:
                    for sc in range(4):
                        us.append(lambda sc=sc: outproj_unit(0, sc))
                if ts == 2:
                    for sc in range(2):
                        us.append(lambda sc=sc: outproj_unit(1, sc))
                if ts == 3:
                    for sc in range(2, 4):
                        us.append(lambda sc=sc: outproj_unit(1, sc))
                    for sc in range(4):
                        us.append(lambda sc=sc: outproj_unit(2, sc))
                if ts + 1 < n_st:
                    for m in range(4):
                        us.append(lambda ts=ts, m=m: proj_k_unit(ts + 1, m))
                    for sc in range(4):
                        us.append(lambda ts=ts, sc=sc: proj_v_unit(ts + 1, sc))
                return us

            # ---- attention: software-pipelined scores/exp -> ctx with PE
            # filler between the steps; the normalize of pair j is emitted
            # after pair j+1's first scores ----
            def attn(qb, filler, defer_final_norm=False):
                nt = 4 * (qb + 1)
                n_steps = 4 * nt
                done = [0]
                step = [0]

                # last stage: drain the filler by ~80% so its tail does not
                # collide with the final output projection
                denom = (n_steps + 8) if qb + 1 < n_st else max(1, n_steps - 12)

                def pop():
                    step[0] += 1
                    want = min(len(filler),
                               (len(filler) * step[0]) // denom)
                    while done[0] < want:
                        filler[done[0]]()
                        done[0] += 1

                def normalize(j, c0, c1):
                    with nc.allow_low_precision(reason="bf16 softmax"):
                        r0 = npool.tile([1, QB], F32, tag="r0", name="r0")
                        r1 = npool.tile([1, QB], F32, tag="r1", name="r1")
                        cs0 = npool.tile([DK, QB], BF16, tag="cs0", name="cs0")
                        cs1 = npool.tile([DK, QB], BF16, tag="cs1", name="cs1")
                        # NOTE: reciprocal_approx_fast (custom DVE op)
                        # returns garbage on actual hardware here -- keep the
                        # plain DVE reciprocal.
                        nc.vector.reciprocal(r0[:], c0[DK:DK + 1, :])
                        nc.vector.reciprocal(r1[:], c1[DK:DK + 1, :])
                        if qb + 1 < n_st:
                            nc.scalar.activation(cs0[:], c0[0:DK, :], AF.Copy,
                                                 bias=0.0, scale=1.0)
                        else:
                            # last stage is ACT-paced: keep the copy off ACT
                            nc.vector.tensor_copy(cs0[:], c0[0:DK, :])
                        nc.vector.tensor_copy(cs1[:], c1[0:DK, :])
                        rb0 = npool.tile([DK, QB], F32, tag="rb0", name="rb0")
                        rb1 = npool.tile([DK, QB], F32, tag="rb1", name="rb1")
                        nc.gpsimd.partition_broadcast(rb0[:], r0[:])
                        nc.gpsimd.partition_broadcast(rb1[:], r1[:])
                        nc.gpsimd.tensor_tensor(
                            ctxT[qb][j][0:64, :], cs0[:], rb0[:],
                            op=mybir.AluOpType.mult)
                        nc.gpsimd.tensor_tensor(
                            ctxT[qb][j][64:128, :], cs1[:], rb1[:],
                            op=mybir.AluOpType.mult)

                pend = [None]
                for j in range(4):          # head pairs
                    h0, h1 = 2 * j, 2 * j + 1
                    e4b = [None] * (nt // 2)
                    lob = [0] * nt

                    def scores2(tp, j=j, e4b=e4b, lob=lob):
                        # two k-tiles' scores into one 4-bank PSUM tile so a
                        # single exp call covers both (halves ACT overhead)
                        s4 = psum.tile([128, 4, QB], F32, tag="sc4",
                                       name=f"s4_{qb}_{j}_{tp}")
                        e4 = epool.tile([128, 4, QB], BF16, tag="e01",
                                        name=f"e4_{qb}_{j}_{tp}")
                        for half in range(2):
                            t = 2 * tp + half
                            tks, tkc = t // 4, t % 4
                            ksl = slice(tkc * KT, (tkc + 1) * KT)
                            jj = t - 4 * qb
                            lo = jj * KT if jj > 0 else 0
                            lob[t] = lo
                            nc.tensor.matmul(
                                s4[:, 2 * half, lo:], khT[tks][j][0:64, ksl],
                                qhT[qb][j][0:64, lo:], start=True, stop=True)
                            nc.tensor.matmul(
                                s4[:, 2 * half + 1, lo:],
                                khT[tks][j][64:128, ksl],
                                qhT[qb][j][64:128, lo:], start=True, stop=True,
                                tile_position=(64, 0))
                        lo_e = lob[2 * tp]
                        nc.scalar.activation(e4[:, :, lo_e:], s4[:, :, lo_e:],
                                             AF.Exp, scale=scale)
                        for half in range(2):
                            t = 2 * tp + half
                            lo = lob[t]
                            if t - 4 * qb >= 0:
                                nc.vector.tensor_mul(
                                    e4[:, 2 * half:2 * half + 2, lo:lo + KT],
                                    e4[:, 2 * half:2 * half + 2, lo:lo + KT],
                                    masks[:].unsqueeze(1).broadcast_to(
                                        [128, 2, KT]))
                        e4b[tp] = e4

                    scores2(0)
                    if pend[0] is not None:
                        pend[0]()
                        pend[0] = None
                    c0 = psum.tile([DK + 1, QB], F32, tag="c0",
                                   name=f"c0_{qb}_{j}")
                    c1 = psum.tile([DK + 1, QB], F32, tag="c1",
                                   name=f"c1_{qb}_{j}")

                    def ctx(t, c0=c0, c1=c1, h0=h0, h1=h1, e4b=e4b, lob=lob):
                        lo = lob[t]
                        e4 = e4b[t // 2]
                        hh = 2 * (t % 2)
                        nc.tensor.matmul(
                            c0[:, lo:], vh[t][:, h0, :], e4[:, hh, lo:],
                            start=(t == 0), stop=(t == nt - 1))
                        nc.tensor.matmul(
                            c1[:, lo:], vh[t][:, h1, :], e4[:, hh + 1, lo:],
                            start=(t == 0), stop=(t == nt - 1))

                    for tp in range(1, nt // 2):
                        scores2(tp)
                        pop()
                        ctx(2 * tp - 2)
                        ctx(2 * tp - 1)
                        pop()
                    pop()
                    ctx(nt - 2)
                    pop()
                    ctx(nt - 1)
                    pend[0] = (lambda j=j, c0=c0, c1=c1: normalize(j, c0, c1))
                # leftover filler first so its PSUM consumers don't queue
                # behind the final normalize on DVE
                while done[0] < len(filler):
                    filler[done[0]]()
                    done[0] += 1
                if defer_final_norm:
                    return pend[0]
                pend[0]()

            # ---- pipeline ----
            proj_stage0()
            for ts in range(n_st):
                if ts + 2 < n_st:
                    stage_x_dma(ts + 2)
                if ts + 1 < n_st:
                    attn(ts, make_filler(ts))
                else:
                    pendn = attn(ts, make_filler(ts), defer_final_norm=True)
            outproj_tail(n_st - 1, pendn)

            psum.release()

    nc.compile()
    return nc


def _get_nc(s=S):
    if s not in _CACHE:
        _CACHE[s] = _build(s)
    return _CACHE[s]


def _make_masks(s=S):
    # triangular strip: valid iff local q index >= local k index
    m = np.zeros((KT, KT), np.float32)
    for kk in range(KT):
        m[kk, kk:] = 1.0
    return m.astype(ml_dtypes.bfloat16)


def make_in_maps(q, k, v, Wq, bq, Wk, bk, Wv, bv, Wo, s=S):
    BF = ml_dtypes.bfloat16
    masks = _make_masks(s)
    qT = [np.ascontiguousarray(q[b].T).astype(BF) for b in range(B)]
    kT = [np.ascontiguousarray(k[b].T).astype(BF) for b in range(B)]
    vT = [np.ascontiguousarray(v[b].T).astype(BF) for b in range(B)]
    in_maps = []
    for c in range(N_CORES):
        b, g = c // 2, c % 2
        gsl = slice(g * O, (g + 1) * O)
        in_maps.append({
            "xqT": qT[b],
            "xkT": kT[b],
            "xvT": vT[b],
            "qpk": np.ascontiguousarray(np.concatenate(
                [Wq[gsl, :].T.astype(BF), qT[b][:, 0:SB]], axis=1)),
            "kpk": np.ascontiguousarray(np.concatenate(
                [Wk[gsl, :].T.astype(BF), kT[b][:, 0:SB]], axis=1)),
            "vpk": np.ascontiguousarray(np.concatenate(
                [Wv[gsl, :].T.astype(BF), vT[b][:, 0:SB]], axis=1)),
            "bq": np.ascontiguousarray(bq[gsl]),
            "bk": np.ascontiguousarray(bk[gsl]),
            "bv_bc": np.ascontiguousarray(
                np.broadcast_to(bv[gsl][None, :], (128, O))).astype(BF),
            "woT": np.ascontiguousarray(Wo[:, gsl].T).astype(BF),
            "masks": masks,
            "ones8": np.ones((128, HPC), ml_dtypes.bfloat16),
        })
    return in_maps


def kernel(q, k, v, mask, Wq, bq, Wk, bk, Wv, bv, Wo, bo):
    q = np.asarray(q, np.float32)
    k = np.asarray(k, np.float32)
    v = np.asarray(v, np.float32)
    nc = _get_nc(S)
    in_maps = make_in_maps(q, k, v,
                           np.asarray(Wq, np.float32), np.asarray(bq, np.float32),
                           np.asarray(Wk, np.float32), np.asarray(bk, np.float32),
                           np.asarray(Wv, np.float32), np.asarray(bv, np.float32),
                           np.asarray(Wo, np.float32), S)
    res = run_bass_kernel_spmd(nc, in_maps, list(range(N_CORES)))
    bo = np.asarray(bo, np.float32)
    out = np.empty((B, S, D), np.float32)
    for b in range(B):
        out[b] = (np.asarray(res.results[2 * b]["out"], np.float32)
                  + np.asarray(res.results[2 * b + 1]["out"], np.float32)
                  + bo)
    return out


# revision 28
# speedup vs baseline: 1.1546x; 1.1546x over previous
"""Multi-head attention Trainium2 kernel (B=4, S=2048, D=1024, H=16, causal).

Sharding: 8 cores = 4 batches x 2 head-groups (8 heads each, tensor-parallel
over the QKV/out projection weights along the head dimension).

Single software-pipelined pass per core (no serial phases): stage ts in 0..3
computes q-block ts of the causal attention; the projections for s-block
ts+1 and the output projection of earlier q-blocks are interleaved into the
(ACT-paced) attention loop as PE filler so the tensor engine never waits on
the exp chain.

  - host supplies transposed activations xT [D, S] and weights in bf16
    (halves DMA; matmuls run at full PE rate either way, accumulation stays
    fp32 in PSUM).  x/w loads are merged into few descriptor-rich DMAs (SP
    DGE config costs 565ns per dma_start); stage-0 loads are laddered in
    d-chunks of (1,1,2,4) so the first matmul starts ~3us in.
  - small loads (biases, masks) go through the GpSimd SWDGE queue.
  - projections produce qhT/khT head-major [o, s] (bias folded into an ACT
    Identity+bias op straight out of PSUM) and vh sequence-major
    [s, (h, dk+1)] with a ones column for the softmax denominator.
  - stage-0 projections run contraction-outer across four PSUM banks so the
    PE streams behind the arriving x DMA chunks.
  - scoresT[k, q] per head pair in one 2-bank PSUM tile; exp on ACT with the
    1/sqrt(dk) scale folded in writes bf16 e01; causal strip masked by a
    bf16 DVE multiply.
  - ctx accumulation per head into [dk+1, q] PSUM; denominator in row 64.
    Normalize (emitted after the next pair's first scores so the exp chain
    never waits): DVE reciprocal_approx_fast on the denominator rows + PSUM
    evacuation copies split across ACT/DVE (frees the accumulator banks
    early), then GpSimd partition_broadcast + GpSimd multiplies.
  - output projection consumes the d'-major bf16 ctxT; the last q-block's
    projection pre-runs its first three weight tiles across six PSUM banks
    while the final normalize drains.  Per-core bf16 partials are summed
    pairwise (+ bo) in fp32 on the host.
"""

import numpy as np
import ml_dtypes

import concourse.bacc as bacc
import concourse.mybir as mybir
import concourse.tile as tile
from concourse.bass_utils import run_bass_kernel_spmd

B, S, D, H = 4, 2048, 1024, 16
DK = D // H          # 64
N_CORES = 8
O = 512              # head dims per core (8 heads x 64)
HPC = 8              # heads per core
SB = 512             # s-block (= stage granularity = q-block)
QB = 512
KT = 128             # k tile
F32 = mybir.dt.float32
BF16 = mybir.dt.bfloat16
AF = mybir.ActivationFunctionType

QCH = [(0, 1), (1, 2), (2, 4), (4, 8)]   # stage-0 d-chunk ladder



_CACHE = {}


def _build(s=S):
    nc = bacc.Bacc("TRN2", target_bir_lowering=False, debug=False,
                   num_devices=N_CORES)
    n_st = s // SB            # pipeline stages / q-blocks / s-blocks
    n_sc = s // 128           # s chunks of 128

    xqT = nc.declare_dram_parameter("xqT", [D, s], BF16, isOutput=False)
    xkT = nc.declare_dram_parameter("xkT", [D, s], BF16, isOutput=False)
    xvT = nc.declare_dram_parameter("xvT", [D, s], BF16, isOutput=False)
    # stage-0 packed loads: row d = [wT[d, :] | xT[d, 0:SB]] so one DMA per
    # d-chunk delivers both the weight tile and the first x block
    qpk = nc.declare_dram_parameter("qpk", [D, O + SB], BF16, isOutput=False)
    kpk = nc.declare_dram_parameter("kpk", [D, O + SB], BF16, isOutput=False)
    vpk = nc.declare_dram_parameter("vpk", [D, O + SB], BF16, isOutput=False)
    bqd = nc.declare_dram_parameter("bq", [O], F32, isOutput=False)
    bkd = nc.declare_dram_parameter("bk", [O], F32, isOutput=False)
    bvb = nc.declare_dram_parameter("bv_bc", [128, O], BF16, isOutput=False)
    wod = nc.declare_dram_parameter("woT", [O, D], BF16, isOutput=False)
    maskd = nc.declare_dram_parameter("masks", [KT, KT], BF16, isOutput=False)
    onesd = nc.declare_dram_parameter("ones8", [128, HPC], BF16, isOutput=False)
    outd = nc.declare_dram_parameter("out", [s, D], BF16, isOutput=True)

    scale = float(DK) ** -0.5

    xq_r = xqT.ap().rearrange("(a p) s -> p a s", p=128)
    xk_r = xkT.ap().rearrange("(a p) s -> p a s", p=128)
    xv_r = xvT.ap().rearrange("(a p) s -> p a s", p=128)
    qpk_r = qpk.ap().rearrange("(a p) o -> p a o", p=128)
    kpk_r = kpk.ap().rearrange("(a p) o -> p a o", p=128)
    vpk_r = vpk.ap().rearrange("(a p) o -> p a o", p=128)
    wo_r = wod.ap().rearrange("(a p) d -> p a d", p=128)

    with tile.TileContext(nc) as tc:
        with (
            tc.tile_pool(name="res", bufs=1) as res,
            tc.tile_pool(name="xpool", bufs=2) as xpool,
            tc.tile_pool(name="epool", bufs=5) as epool,
            tc.tile_pool(name="npool", bufs=1) as npool,
            tc.tile_pool(name="outpool", bufs=4) as outpool,
        ):
            psum = tc.alloc_tile_pool(name="psum", bufs=1, space="PSUM")

            # ---- persistent tiles ----
            qhT = [[res.tile([128, SB], BF16, tag=f"qhT{ts}_{j}",
                             name=f"qhT{ts}_{j}") for j in range(4)]
                   for ts in range(n_st)]
            khT = [[res.tile([128, SB], BF16, tag=f"khT{ts}_{j}",
                             name=f"khT{ts}_{j}") for j in range(4)]
                   for ts in range(n_st)]
            vh = [res.tile([128, HPC, DK + 1], BF16, tag=f"vh{i}",
                           name=f"vh{i}") for i in range(n_sc)]
            ctxT = [[res.tile([128, SB], BF16, tag=f"ctxT{ts}_{j}",
                              name=f"ctxT{ts}_{j}") for j in range(4)]
                    for ts in range(n_st)]
            wq_c = [res.tile([128, e - b, O + SB], BF16, tag=f"wqc{i}",
                             name=f"wqc{i}") for i, (b, e) in enumerate(QCH)]
            wk_m = res.tile([128, 8, O + SB], BF16, tag="wk_m", name="wk_m")
            wv_m = res.tile([128, 8, O + SB], BF16, tag="wv_m", name="wv_m")
            wo_m = res.tile([128, 4, D], BF16, tag="wo_m", name="wo_m")
            bq_t = res.tile([128, O // 128], F32, tag="bq_t", name="bq_t")
            bk_t = res.tile([128, O // 128], F32, tag="bk_t", name="bk_t")
            bv_t = res.tile([128, O], BF16, tag="bv_t", name="bv_t")
            masks = res.tile([128, KT], BF16, tag="masks", name="masks")

            def wq_sl(d, csl):
                for i, (b, e) in enumerate(QCH):
                    if b <= d < e:
                        return wq_c[i][:, d - b, csl]
                raise AssertionError

            # ---- small loads via the GpSimd SWDGE queue (25ns config);
            # the first stage-0 chunk jumps this idle queue to cut startup ----
            small_eng = nc.gpsimd
            small_eng.dma_start(wq_c[0][:], qpk_r[:, 0:1, :])
            small_eng.dma_start(
                bq_t[:], bqd.ap().rearrange("(m p) -> p m", p=128))
            small_eng.dma_start(
                bk_t[:], bkd.ap().rearrange("(m p) -> p m", p=128))
            small_eng.dma_start(bv_t[:], bvb[:, :])
            small_eng.dma_start(masks[:], maskd[:, :])
            for i in range(n_sc):
                nc.vector.memset(vh[i][:, :, DK], 1.0)

            # ---- bulk loads on SP, laddered for stage-0 streaming ----
            def xq0_sl(d):
                for i, (b, e) in enumerate(QCH):
                    if b <= d < e:
                        return wq_c[i][:, d - b, O:O + SB]
                raise AssertionError

            for i, (b, e) in enumerate(QCH):
                if i > 0:
                    nc.sync.dma_start(wq_c[i][:], qpk_r[:, b:e, :])
                if i == 2:
                    nc.sync.dma_start(wk_m[:, 0:4, :], kpk_r[:, 0:4, :])
            nc.sync.dma_start(wk_m[:, 4:8, :], kpk_r[:, 4:8, :])
            nc.sync.dma_start(wv_m[:], vpk_r[:, :, :])

            xq_b = [None] * n_st
            xk_b = [None] * n_st
            xv_b = [None] * n_st

            def stage_x_dma(ts):
                ssl = slice(ts * SB, (ts + 1) * SB)
                xq_b[ts] = xpool.tile([128, 8, SB], BF16, tag="xqm",
                                      name=f"xq{ts}")
                nc.sync.dma_start(xq_b[ts][:], xq_r[:, :, ssl])
                xk_b[ts] = xpool.tile([128, 8, SB], BF16, tag="xkm",
                                      name=f"xk{ts}")
                nc.sync.dma_start(xk_b[ts][:], xk_r[:, :, ssl])
                xv_b[ts] = xpool.tile([128, 8, SB], BF16, tag="xvm",
                                      name=f"xv{ts}")
                nc.sync.dma_start(xv_b[ts][:], xv_r[:, :, ssl])

            if n_st > 1:
                stage_x_dma(1)
            nc.sync.dma_start(wo_m[:], wo_r[:, :, :])

            # ---- stage-0 projections, contraction-outer, with the q/k/v
            # phases striped across different PSUM banks so no phase waits
            # on the previous phase's consumers ----
            def proj_stage0():
                t4 = ["f0", "f1", "sc0", "sc1"]
                t4k = ["c0", "c1", "f0", "f1"]
                t4v = ["sc0", "sc1", "c0", "c1"]
                psq = [psum.tile([128, SB], F32, tag=t4[m], name=f"p0q{m}")
                       for m in range(4)]
                for d in range(8):
                    for m in range(4):
                        nc.tensor.matmul(
                            psq[m][:], wq_sl(d, slice(m * 128, (m + 1) * 128)),
                            xq0_sl(d), start=(d == 0), stop=(d == 7))
                for m in range(4):
                    nc.scalar.activation(qhT[0][m][:], psq[m][:], AF.Identity,
                                         bias=bq_t[:, m:m + 1], scale=1.0)
                psk = [psum.tile([128, SB], F32, tag=t4k[m], name=f"p0k{m}")
                       for m in range(4)]
                for d in range(8):
                    for m in range(4):
                        nc.tensor.matmul(
                            psk[m][:], wk_m[:, d, m * 128:(m + 1) * 128],
                            wk_m[:, d, O:O + SB], start=(d == 0), stop=(d == 7))
                for m in range(4):
                    nc.scalar.activation(khT[0][m][:], psk[m][:], AF.Identity,
                                         bias=bk_t[:, m:m + 1], scale=1.0)
                psv = [psum.tile([128, O], F32, tag=t4v[sc], name=f"p0v{sc}")
                       for sc in range(4)]
                for d in range(8):
                    for sc in range(4):
                        nc.tensor.matmul(
                            psv[sc][:],
                            wv_m[:, d, O + sc * 128:O + (sc + 1) * 128],
                            wv_m[:, d, 0:O], start=(d == 0), stop=(d == 7))
                for sc in range(4):
                    nc.vector.tensor_tensor(
                        vh[sc][:, :, 0:DK],
                        psv[sc][:].rearrange("p (h e) -> p h e", e=DK),
                        bv_t[:].rearrange("p (h e) -> p h e", e=DK),
                        op=mybir.AluOpType.add)

            # ---- filler units (run interleaved inside the attention) ----
            fctr = [0]

            def proj_q_unit(ts, m):
                ps = psum.tile([128, SB], F32, tag=f"f{fctr[0] % 2}",
                               name=f"psq{ts}_{m}")
                fctr[0] += 1
                for d in range(8):
                    nc.tensor.matmul(
                        ps[:], wq_sl(d, slice(m * 128, (m + 1) * 128)),
                        xq_b[ts][:, d, :], start=(d == 0), stop=(d == 7))
                nc.scalar.activation(qhT[ts][m][:], ps[:], AF.Identity,
                                     bias=bq_t[:, m:m + 1], scale=1.0)

            def proj_k_unit(ts, m):
                ps = psum.tile([128, SB], F32, tag=f"f{fctr[0] % 2}",
                               name=f"psk{ts}_{m}")
                fctr[0] += 1
                for d in range(8):
                    nc.tensor.matmul(
                        ps[:], wk_m[:, d, m * 128:(m + 1) * 128],
                        xk_b[ts][:, d, :], start=(d == 0), stop=(d == 7))
                nc.scalar.activation(khT[ts][m][:], ps[:], AF.Identity,
                                     bias=bk_t[:, m:m + 1], scale=1.0)

            def proj_v_unit(ts, sc):
                si = ts * (SB // 128) + sc
                ps = psum.tile([128, O], F32, tag=f"f{fctr[0] % 2}",
                               name=f"psv{ts}_{sc}")
                fctr[0] += 1
                for d in range(8):
                    nc.tensor.matmul(
                        ps[:], xv_b[ts][:, d, sc * 128:(sc + 1) * 128],
                        wv_m[:, d, 0:O], start=(d == 0), stop=(d == 7))
                nc.vector.tensor_tensor(
                    vh[si][:, :, 0:DK],
                    ps[:].rearrange("p (h e) -> p h e", e=DK),
                    bv_t[:].rearrange("p (h e) -> p h e", e=DK),
                    op=mybir.AluOpType.add)

            def outproj_unit(qb, sc):
                ot = outpool.tile([128, D], BF16, tag="out_t", name="ot")
                for oc in range(2):
                    osl = slice(oc * 512, (oc + 1) * 512)
                    ps = psum.tile([128, 512], F32, tag=f"f{fctr[0] % 2}",
                                   name=f"pso{qb}_{sc}_{oc}")
                    fctr[0] += 1
                    for jw in range(4):
                        nc.tensor.matmul(
                            ps[:], ctxT[qb][jw][:, sc * 128:(sc + 1) * 128],
                            wo_m[:, jw, osl],
                            start=(jw == 0), stop=(jw == 3))
                    nc.vector.tensor_copy(ot[:, osl], ps[:])
                sg = qb * (SB // 128) + sc
                nc.sync.dma_start(outd[sg * 128:(sg + 1) * 128, :], ot[:])

            def outproj_tail(qb, pendn):
                """Final q-block's projection: pre-run the first three
                weight tiles of four (sc, oc) groups on banks the final
                normalize does not read, emit the deferred normalize, then
                finish."""
                tpre = ["f0", "f1", "sc0", "sc1"]
                trest = ["c0", "c1", "f0", "f1"]
                groups = [(sc, oc) for sc in range(4) for oc in range(2)]
                ots = [outpool.tile([128, D], BF16, tag="out_t",
                                    name=f"ott{sc}") for sc in range(4)]
                pss = {}
                for gi, (sc, oc) in enumerate(groups[:4]):
                    osl = slice(oc * 512, (oc + 1) * 512)
                    ps = psum.tile([128, 512], F32, tag=tpre[gi],
                                   name=f"pst{sc}_{oc}")
                    pss[(sc, oc)] = ps
                    for jw in range(3):
                        nc.tensor.matmul(
                            ps[:], ctxT[qb][jw][:, sc * 128:(sc + 1) * 128],
                            wo_m[:, jw, osl],
                            start=(jw == 0), stop=False)
                def tail_copy(sc, oc, ps):
                    osl = slice(oc * 512, (oc + 1) * 512)
                    if oc == 0:
                        nc.scalar.activation(ots[sc][:, osl], ps[:], AF.Copy,
                                             bias=0.0, scale=1.0)
                    else:
                        nc.vector.tensor_copy(ots[sc][:, osl], ps[:])

                def tail_dma(sc, osl=slice(0, D)):
                    sg = qb * (SB // 128) + sc
                    eng = nc.scalar if sc % 2 == 0 else nc.sync
                    eng.dma_start(outd[sg * 128:(sg + 1) * 128, osl],
                                  ots[sc][:, osl])

                pendn()
                for sc, oc in groups[:4]:
                    osl = slice(oc * 512, (oc + 1) * 512)
                    ps = pss[(sc, oc)]
                    nc.tensor.matmul(
                        ps[:], ctxT[qb][3][:, sc * 128:(sc + 1) * 128],
                        wo_m[:, 3, osl], start=False, stop=True)
                    tail_copy(sc, oc, ps)
                    if oc == 1:
                        tail_dma(sc)
                for gi, (sc, oc) in enumerate(groups[4:]):
                    osl = slice(oc * 512, (oc + 1) * 512)
                    ps = psum.tile([128, 512], F32, tag=trest[gi],
                                   name=f"pst2_{sc}_{oc}")
                    for jw in range(4):
                        nc.tensor.matmul(
                            ps[:], ctxT[qb][jw][:, sc * 128:(sc + 1) * 128],
                            wo_m[:, jw, osl],
                            start=(jw == 0), stop=(jw == 3))
                    tail_copy(sc, oc, ps)
                    if sc < 3 and oc == 1:
                        tail_dma(sc)
                    elif sc == 3:
                        tail_dma(sc, osl)

            def make_filler(ts):
                us = []
                if ts + 1 < n_st:
                    for m in range(4):
                        us.append(lambda ts=ts, m=m: proj_q_unit(ts + 1, m))
                if ts == 1:
                    for sc in range(4):
                        us.append(lambda sc=sc: outproj_unit(0, sc))
                if ts == 2:
                    for sc in range(2):
                        us.append(lambda sc=sc: outproj_unit(1, sc))
                if ts == 3:
                    for sc in range(2, 4):
                        us.append(lambda sc=sc: outproj_unit(1, sc))
                    for sc in range(4):
                        us.append(lambda sc=sc: outproj_unit(2, sc))
                if ts + 1 < n_st:
                    for m in range(4):
                        us.append(lambda ts=ts, m=m: proj_k_unit(ts + 1, m))
                    for sc in range(4):
                        us.append(lambda ts=ts, sc=sc: proj_v_unit(ts + 1, sc))
                return us

            # ---- attention: software-pipelined scores/exp -> ctx with PE
            # filler between the steps; the normalize of pair j is emitted
            # after pair j+1's first scores ----
            def attn(qb, filler, defer_final_norm=False):
                nt = 4 * (qb + 1)
                n_steps = 4 * nt
                done = [0]
                step = [0]

                # last stage: drain the filler by ~80% so its tail does not
                # collide with the final output projection
                denom = (n_steps + 8) if qb + 1 < 4 else max(1, n_steps - 12)

                def pop():
                    step[0] += 1
                    want = min(len(filler),
                               (len(filler) * step[0]) // denom)
                    while done[0] < want:
                        filler[done[0]]()
                        done[0] += 1

                def normalize(j, c0, c1):
                    with nc.allow_low_precision(reason="bf16 softmax"):
                        r0 = npool.tile([1, QB], F32, tag="r0", name="r0")
                        r1 = npool.tile([1, QB], F32, tag="r1", name="r1")
                        cs0 = npool.tile([DK, QB], BF16, tag="cs0", name="cs0")
                        cs1 = npool.tile([DK, QB], BF16, tag="cs1", name="cs1")
                        # NOTE: reciprocal_approx_fast (custom DVE op)
                        # returns garbage on actual hardware here -- keep the
                        # plain DVE reciprocal.
                        nc.vector.reciprocal(r0[:], c0[DK:DK + 1, :])
                        nc.vector.reciprocal(r1[:], c1[DK:DK + 1, :])
                        if qb + 1 < 4:
                            nc.scalar.activation(cs0[:], c0[0:DK, :], AF.Copy,
                                                 bias=0.0, scale=1.0)
                        else:
                            # last stage is ACT-paced: keep the copy off ACT
                            nc.vector.tensor_copy(cs0[:], c0[0:DK, :])
                        nc.vector.tensor_copy(cs1[:], c1[0:DK, :])
                        rb0 = npool.tile([DK, QB], F32, tag="rb0", name="rb0")
                        rb1 = npool.tile([DK, QB], F32, tag="rb1", name="rb1")
                        nc.gpsimd.partition_broadcast(rb0[:], r0[:])
                        nc.gpsimd.partition_broadcast(rb1[:], r1[:])
                        nc.gpsimd.tensor_tensor(
                            ctxT[qb][j][0:64, :], cs0[:], rb0[:],
                            op=mybir.AluOpType.mult)
                        nc.gpsimd.tensor_tensor(
                            ctxT[qb][j][64:128, :], cs1[:], rb1[:],
                            op=mybir.AluOpType.mult)

                pend = [None]
                for j in range(4):          # head pairs
                    h0, h1 = 2 * j, 2 * j + 1
                    eb = [None] * nt
                    lob = [0] * nt

                    def scores(t, j=j, eb=eb, lob=lob):
                        tks, tkc = t // 4, t % 4
                        ksl = slice(tkc * KT, (tkc + 1) * KT)
                        jj = t - 4 * qb
                        lo = jj * KT if jj > 0 else 0
                        lob[t] = lo
                        s01 = psum.tile([128, 2, QB], F32, tag=f"sc{t % 2}",
                                        name=f"s01_{qb}_{j}_{t}")
                        nc.tensor.matmul(
                            s01[:, 0, lo:], khT[tks][j][0:64, ksl],
                            qhT[qb][j][0:64, lo:], start=True, stop=True)
                        nc.tensor.matmul(
                            s01[:, 1, lo:], khT[tks][j][64:128, ksl],
                            qhT[qb][j][64:128, lo:], start=True, stop=True,
                            tile_position=(64, 0))
                        e01 = epool.tile([128, 2, QB], BF16, tag="e01",
                                         name=f"e01_{qb}_{j}_{t}")
                        nc.scalar.activation(e01[:, :, lo:], s01[:, :, lo:],
                                             AF.Exp, scale=scale)
                        if jj >= 0:
                            nc.vector.tensor_mul(
                                e01[:, :, lo:lo + KT], e01[:, :, lo:lo + KT],
                                masks[:].unsqueeze(1).broadcast_to(
                                    [128, 2, KT]))
                        eb[t] = e01

                    scores(0)
                    if pend[0] is not None:
                        pend[0]()
                        pend[0] = None
                    c0 = psum.tile([DK + 1, QB], F32, tag="c0",
                                   name=f"c0_{qb}_{j}")
                    c1 = psum.tile([DK + 1, QB], F32, tag="c1",
                                   name=f"c1_{qb}_{j}")

                    def ctx(t, c0=c0, c1=c1, h0=h0, h1=h1, eb=eb, lob=lob):
                        lo = lob[t]
                        nc.tensor.matmul(
                            c0[:, lo:], vh[t][:, h0, :], eb[t][:, 0, lo:],
                            start=(t == 0), stop=(t == nt - 1))
                        nc.tensor.matmul(
                            c1[:, lo:], vh[t][:, h1, :], eb[t][:, 1, lo:],
                            start=(t == 0), stop=(t == nt - 1))

                    for t in range(1, nt):
                        scores(t)
                        pop()
                        ctx(t - 1)
                    pop()
                    ctx(nt - 1)
                    pend[0] = (lambda j=j, c0=c0, c1=c1: normalize(j, c0, c1))
                # leftover filler first so its PSUM consumers don't queue
                # behind the final normalize on DVE
                while done[0] < len(filler):
                    filler[done[0]]()
                    done[0] += 1
                if defer_final_norm:
                    return pend[0]
                pend[0]()

            # ---- pipeline ----
            proj_stage0()
            pendn = None
            for ts in range(n_st):
                if ts + 2 < n_st:
                    stage_x_dma(ts + 2)
                if ts + 1 < n_st:
                    attn(ts, make_filler(ts))
                else:
                    pendn = attn(ts, make_filler(ts), defer_final_norm=True)
            outproj_tail(n_st - 1, pendn)

            psum.release()

    nc.compile()
    return nc


def _get_nc(s=S):
    if s not in _CACHE:
        _CACHE[s] = _build(s)
    return _CACHE[s]


def _make_masks(s=S):
    # triangular strip: valid iff local q index >= local k index
    m = np.zeros((KT, KT), np.float32)
    for kk in range(KT):
        m[kk, kk:] = 1.0
    return m.astype(ml_dtypes.bfloat16)


def make_in_maps(q, k, v, Wq, bq, Wk, bk, Wv, bv, Wo, s=S):
    BF = ml_dtypes.bfloat16
    masks = _make_masks(s)
    qT = [np.ascontiguousarray(q[b].T).astype(BF) for b in range(B)]
    kT = [np.ascontiguousarray(k[b].T).astype(BF) for b in range(B)]
    vT = [np.ascontiguousarray(v[b].T).astype(BF) for b in range(B)]
    in_maps = []
    for c in range(N_CORES):
        b, g = c // 2, c % 2
        gsl = slice(g * O, (g + 1) * O)
        in_maps.append({
            "xqT": qT[b],
            "xkT": kT[b],
            "xvT": vT[b],
            "qpk": np.ascontiguousarray(np.concatenate(
                [Wq[gsl, :].T.astype(BF), qT[b][:, 0:SB]], axis=1)),
            "kpk": np.ascontiguousarray(np.concatenate(
                [Wk[gsl, :].T.astype(BF), kT[b][:, 0:SB]], axis=1)),
            "vpk": np.ascontiguousarray(np.concatenate(
                [Wv[gsl, :].T.astype(BF), vT[b][:, 0:SB]], axis=1)),
            "bq": np.ascontiguousarray(bq[gsl]),
            "bk": np.ascontiguousarray(bk[gsl]),
            "bv_bc": np.ascontiguousarray(
                np.broadcast_to(bv[gsl][None, :], (128, O))).astype(BF),
            "woT": np.ascontiguousarray(Wo[:, gsl].T).astype(BF),
            "masks": masks,
            "ones8": np.ones((128, HPC), ml_dtypes.bfloat16),
        })
    return in_maps


def kernel(q, k, v, mask, Wq, bq, Wk, bk, Wv, bv, Wo, bo):
    q = np.asarray(q, np.float32)
    k = np.asarray(k, np.float32)
    v = np.asarray(v, np.float32)
    nc = _get_nc(S)
    in_maps = make_in_maps(q, k, v,
                           np.asarray(Wq, np.float32), np.asarray(bq, np.float32),
                           np.asarray(Wk, np.float32), np.asarray(bk, np.float32),
                           np.asarray(Wv, np.float32), np.asarray(bv, np.float32),
                           np.asarray(Wo, np.float32), S)
    res = run_bass_kernel_spmd(nc, in_maps, list(range(N_CORES)))
    bo = np.asarray(bo, np.float32)
    out = np.empty((B, S, D), np.float32)
    for b in range(B):
        out[b] = (np.asarray(res.results[2 * b]["out"], np.float32)
                  + np.asarray(res.results[2 * b + 1]["out"], np.float32)
                  + bo)
    return out


# revision 29
# speedup vs baseline: 1.1647x; 1.0088x over previous
"""Multi-head attention Trainium2 kernel (B=4, S=2048, D=1024, H=16, causal).

Sharding: 8 cores = 4 batches x 2 head-groups (8 heads each, tensor-parallel
over the QKV/out projection weights along the head dimension).

Single software-pipelined pass per core (no serial phases): stage ts in 0..3
computes q-block ts of the causal attention; the projections for s-block
ts+1 and the output projection of earlier q-blocks are interleaved into the
(ACT-paced) attention loop as PE filler so the tensor engine never waits on
the exp chain.

  - host supplies transposed activations xT [D, S] and weights in bf16
    (halves DMA; matmuls run at full PE rate either way, accumulation stays
    fp32 in PSUM).  x/w loads are merged into few descriptor-rich DMAs (SP
    DGE config costs 565ns per dma_start); stage-0 loads are laddered in
    d-chunks of (1,1,2,4) so the first matmul starts ~3us in.
  - small loads (biases, masks) go through the GpSimd SWDGE queue.
  - projections produce qhT/khT head-major [o, s] (bias folded into an ACT
    Identity+bias op straight out of PSUM) and vh sequence-major
    [s, (h, dk+1)] with a ones column for the softmax denominator.
  - stage-0 projections run contraction-outer across four PSUM banks so the
    PE streams behind the arriving x DMA chunks.
  - scoresT[k, q] per head pair in one 2-bank PSUM tile; exp on ACT with the
    1/sqrt(dk) scale folded in writes bf16 e01; causal strip masked by a
    bf16 DVE multiply.
  - ctx accumulation per head into [dk+1, q] PSUM; denominator in row 64.
    Normalize (emitted after the next pair's first scores so the exp chain
    never waits): DVE reciprocal_approx_fast on the denominator rows + PSUM
    evacuation copies split across ACT/DVE (frees the accumulator banks
    early), then GpSimd partition_broadcast + GpSimd multiplies.
  - output projection consumes the d'-major bf16 ctxT; the last q-block's
    projection pre-runs its first three weight tiles across six PSUM banks
    while the final normalize drains.  Per-core bf16 partials are summed
    pairwise (+ bo) in fp32 on the host.
"""

import numpy as np
import ml_dtypes

import concourse.bacc as bacc
import concourse.mybir as mybir
import concourse.tile as tile
from concourse.bass_utils import run_bass_kernel_spmd

B, S, D, H = 4, 2048, 1024, 16
DK = D // H          # 64
N_CORES = 8
O = 512              # head dims per core (8 heads x 64)
HPC = 8              # heads per core
SB = 512             # s-block (= stage granularity = q-block)
QB = 512
KT = 128             # k tile
F32 = mybir.dt.float32
BF16 = mybir.dt.bfloat16
AF = mybir.ActivationFunctionType

QCH = [(0, 1), (1, 2), (2, 4), (4, 8)]   # stage-0 d-chunk ladder



_CACHE = {}


def _build(s=S):
    nc = bacc.Bacc("TRN2", target_bir_lowering=False, debug=False,
                   num_devices=N_CORES)
    n_st = s // SB            # pipeline stages / q-blocks / s-blocks
    n_sc = s // 128           # s chunks of 128

    xqT = nc.declare_dram_parameter("xqT", [D, s], BF16, isOutput=False)
    xkT = nc.declare_dram_parameter("xkT", [D, s], BF16, isOutput=False)
    xvT = nc.declare_dram_parameter("xvT", [D, s], BF16, isOutput=False)
    # stage-0 packed loads: row d = [wT[d, :] | xT[d, 0:SB]] so one DMA per
    # d-chunk delivers both the weight tile and the first x block
    qpk = nc.declare_dram_parameter("qpk", [D, O + SB], BF16, isOutput=False)
    kpk = nc.declare_dram_parameter("kpk", [D, O + SB], BF16, isOutput=False)
    vpk = nc.declare_dram_parameter("vpk", [D, O + SB], BF16, isOutput=False)
    bqd = nc.declare_dram_parameter("bq", [O], F32, isOutput=False)
    bkd = nc.declare_dram_parameter("bk", [O], F32, isOutput=False)
    bvb = nc.declare_dram_parameter("bv_bc", [128, O], BF16, isOutput=False)
    wod = nc.declare_dram_parameter("woT", [O, D], BF16, isOutput=False)
    maskd = nc.declare_dram_parameter("masks", [KT, KT], BF16, isOutput=False)
    onesd = nc.declare_dram_parameter("ones8", [128, HPC], BF16, isOutput=False)
    outd = nc.declare_dram_parameter("out", [s, D], BF16, isOutput=True)

    scale = float(DK) ** -0.5

    xq_r = xqT.ap().rearrange("(a p) s -> p a s", p=128)
    xk_r = xkT.ap().rearrange("(a p) s -> p a s", p=128)
    xv_r = xvT.ap().rearrange("(a p) s -> p a s", p=128)
    qpk_r = qpk.ap().rearrange("(a p) o -> p a o", p=128)
    kpk_r = kpk.ap().rearrange("(a p) o -> p a o", p=128)
    vpk_r = vpk.ap().rearrange("(a p) o -> p a o", p=128)
    wo_r = wod.ap().rearrange("(a p) d -> p a d", p=128)

    with tile.TileContext(nc) as tc:
        with (
            tc.tile_pool(name="res", bufs=1) as res,
            tc.tile_pool(name="xpool", bufs=2) as xpool,
            tc.tile_pool(name="epool", bufs=5) as epool,
            tc.tile_pool(name="npool", bufs=1) as npool,
            tc.tile_pool(name="outpool", bufs=4) as outpool,
        ):
            psum = tc.alloc_tile_pool(name="psum", bufs=1, space="PSUM")

            # ---- persistent tiles ----
            qhT = [[res.tile([128, SB], BF16, tag=f"qhT{ts}_{j}",
                             name=f"qhT{ts}_{j}") for j in range(4)]
                   for ts in range(n_st)]
            khT = [[res.tile([128, SB], BF16, tag=f"khT{ts}_{j}",
                             name=f"khT{ts}_{j}") for j in range(4)]
                   for ts in range(n_st)]
            vh = [res.tile([128, HPC, DK + 1], BF16, tag=f"vh{i}",
                           name=f"vh{i}") for i in range(n_sc)]
            ctxT = [[res.tile([128, SB], BF16, tag=f"ctxT{ts}_{j}",
                              name=f"ctxT{ts}_{j}") for j in range(4)]
                    for ts in range(n_st)]
            wq_c = [res.tile([128, e - b, O + SB], BF16, tag=f"wqc{i}",
                             name=f"wqc{i}") for i, (b, e) in enumerate(QCH)]
            wk_m = res.tile([128, 8, O + SB], BF16, tag="wk_m", name="wk_m")
            wv_m = res.tile([128, 8, O + SB], BF16, tag="wv_m", name="wv_m")
            wo_m = res.tile([128, 4, D], BF16, tag="wo_m", name="wo_m")
            bq_t = res.tile([128, O // 128], F32, tag="bq_t", name="bq_t")
            bk_t = res.tile([128, O // 128], F32, tag="bk_t", name="bk_t")
            bv_t = res.tile([128, O], BF16, tag="bv_t", name="bv_t")
            masks = res.tile([128, KT], BF16, tag="masks", name="masks")

            def wq_sl(d, csl):
                for i, (b, e) in enumerate(QCH):
                    if b <= d < e:
                        return wq_c[i][:, d - b, csl]
                raise AssertionError

            # ---- small loads via the GpSimd SWDGE queue (25ns config) ----
            small_eng = nc.gpsimd
            small_eng.dma_start(
                bq_t[:], bqd.ap().rearrange("(m p) -> p m", p=128))
            small_eng.dma_start(
                bk_t[:], bkd.ap().rearrange("(m p) -> p m", p=128))
            small_eng.dma_start(bv_t[:], bvb[:, :])
            small_eng.dma_start(masks[:], maskd[:, :])
            for i in range(n_sc):
                nc.vector.memset(vh[i][:, :, DK], 1.0)

            # ---- bulk loads on SP, laddered for stage-0 streaming ----
            def xq0_sl(d):
                for i, (b, e) in enumerate(QCH):
                    if b <= d < e:
                        return wq_c[i][:, d - b, O:O + SB]
                raise AssertionError

            for i, (b, e) in enumerate(QCH):
                nc.sync.dma_start(wq_c[i][:], qpk_r[:, b:e, :])
                if i == 2:
                    nc.sync.dma_start(wk_m[:, 0:4, :], kpk_r[:, 0:4, :])
            nc.sync.dma_start(wk_m[:, 4:8, :], kpk_r[:, 4:8, :])
            nc.sync.dma_start(wv_m[:], vpk_r[:, :, :])

            xq_b = [None] * n_st
            xk_b = [None] * n_st
            xv_b = [None] * n_st

            def stage_x_dma(ts):
                ssl = slice(ts * SB, (ts + 1) * SB)
                xq_b[ts] = xpool.tile([128, 8, SB], BF16, tag="xqm",
                                      name=f"xq{ts}")
                nc.sync.dma_start(xq_b[ts][:], xq_r[:, :, ssl])
                xk_b[ts] = xpool.tile([128, 8, SB], BF16, tag="xkm",
                                      name=f"xk{ts}")
                nc.sync.dma_start(xk_b[ts][:], xk_r[:, :, ssl])
                xv_b[ts] = xpool.tile([128, 8, SB], BF16, tag="xvm",
                                      name=f"xv{ts}")
                nc.sync.dma_start(xv_b[ts][:], xv_r[:, :, ssl])

            if n_st > 1:
                stage_x_dma(1)
            nc.sync.dma_start(wo_m[:], wo_r[:, :, :])

            # ---- stage-0 projections, contraction-outer, with the q/k/v
            # phases striped across different PSUM banks so no phase waits
            # on the previous phase's consumers ----
            def proj_stage0():
                t4 = ["f0", "f1", "sc0", "sc1"]
                t4k = ["c0", "c1", "f0", "f1"]
                t4v = ["sc0", "sc1", "c0", "c1"]
                psq = [psum.tile([128, SB], F32, tag=t4[m], name=f"p0q{m}")
                       for m in range(4)]
                for d in range(8):
                    for m in range(4):
                        nc.tensor.matmul(
                            psq[m][:], wq_sl(d, slice(m * 128, (m + 1) * 128)),
                            xq0_sl(d), start=(d == 0), stop=(d == 7))
                for m in range(4):
                    nc.scalar.activation(qhT[0][m][:], psq[m][:], AF.Identity,
                                         bias=bq_t[:, m:m + 1], scale=1.0)
                psk = [psum.tile([128, SB], F32, tag=t4k[m], name=f"p0k{m}")
                       for m in range(4)]
                for d in range(8):
                    for m in range(4):
                        nc.tensor.matmul(
                            psk[m][:], wk_m[:, d, m * 128:(m + 1) * 128],
                            wk_m[:, d, O:O + SB], start=(d == 0), stop=(d == 7))
                for m in range(4):
                    nc.scalar.activation(khT[0][m][:], psk[m][:], AF.Identity,
                                         bias=bk_t[:, m:m + 1], scale=1.0)
                psv = [psum.tile([128, O], F32, tag=t4v[sc], name=f"p0v{sc}")
                       for sc in range(4)]
                for d in range(8):
                    for sc in range(4):
                        nc.tensor.matmul(
                            psv[sc][:],
                            wv_m[:, d, O + sc * 128:O + (sc + 1) * 128],
                            wv_m[:, d, 0:O], start=(d == 0), stop=(d == 7))
                for sc in range(4):
                    nc.vector.tensor_tensor(
                        vh[sc][:, :, 0:DK],
                        psv[sc][:].rearrange("p (h e) -> p h e", e=DK),
                        bv_t[:].rearrange("p (h e) -> p h e", e=DK),
                        op=mybir.AluOpType.add)

            # ---- filler units (run interleaved inside the attention) ----
            fctr = [0]

            def proj_q_unit(ts, m):
                ps = psum.tile([128, SB], F32, tag=f"f{fctr[0] % 2}",
                               name=f"psq{ts}_{m}")
                fctr[0] += 1
                for d in range(8):
                    nc.tensor.matmul(
                        ps[:], wq_sl(d, slice(m * 128, (m + 1) * 128)),
                        xq_b[ts][:, d, :], start=(d == 0), stop=(d == 7))
                nc.scalar.activation(qhT[ts][m][:], ps[:], AF.Identity,
                                     bias=bq_t[:, m:m + 1], scale=1.0)

            def proj_k_unit(ts, m):
                ps = psum.tile([128, SB], F32, tag=f"f{fctr[0] % 2}",
                               name=f"psk{ts}_{m}")
                fctr[0] += 1
                for d in range(8):
                    nc.tensor.matmul(
                        ps[:], wk_m[:, d, m * 128:(m + 1) * 128],
                        xk_b[ts][:, d, :], start=(d == 0), stop=(d == 7))
                nc.scalar.activation(khT[ts][m][:], ps[:], AF.Identity,
                                     bias=bk_t[:, m:m + 1], scale=1.0)

            def proj_v_unit(ts, sc):
                si = ts * (SB // 128) + sc
                ps = psum.tile([128, O], F32, tag=f"f{fctr[0] % 2}",
                               name=f"psv{ts}_{sc}")
                fctr[0] += 1
                for d in range(8):
                    nc.tensor.matmul(
                        ps[:], xv_b[ts][:, d, sc * 128:(sc + 1) * 128],
                        wv_m[:, d, 0:O], start=(d == 0), stop=(d == 7))
                nc.vector.tensor_tensor(
                    vh[si][:, :, 0:DK],
                    ps[:].rearrange("p (h e) -> p h e", e=DK),
                    bv_t[:].rearrange("p (h e) -> p h e", e=DK),
                    op=mybir.AluOpType.add)

            def outproj_unit(qb, sc):
                ot = outpool.tile([128, D], BF16, tag="out_t", name="ot")
                for oc in range(2):
                    osl = slice(oc * 512, (oc + 1) * 512)
                    ps = psum.tile([128, 512], F32, tag=f"f{fctr[0] % 2}",
                                   name=f"pso{qb}_{sc}_{oc}")
                    fctr[0] += 1
                    for jw in range(4):
                        nc.tensor.matmul(
                            ps[:], ctxT[qb][jw][:, sc * 128:(sc + 1) * 128],
                            wo_m[:, jw, osl],
                            start=(jw == 0), stop=(jw == 3))
                    nc.vector.tensor_copy(ot[:, osl], ps[:])
                sg = qb * (SB // 128) + sc
                nc.sync.dma_start(outd[sg * 128:(sg + 1) * 128, :], ot[:])

            def outproj_tail(qb, pendn):
                """Final q-block's projection: pre-run the first three
                weight tiles of four (sc, oc) groups on banks the final
                normalize does not read, emit the deferred normalize, then
                finish."""
                tpre = ["f0", "f1", "sc0", "sc1"]
                trest = ["c0", "c1", "f0", "f1"]
                groups = [(sc, oc) for sc in range(4) for oc in range(2)]
                ots = [outpool.tile([128, D], BF16, tag="out_t",
                                    name=f"ott{sc}") for sc in range(4)]
                pss = {}
                for gi, (sc, oc) in enumerate(groups[:4]):
                    osl = slice(oc * 512, (oc + 1) * 512)
                    ps = psum.tile([128, 512], F32, tag=tpre[gi],
                                   name=f"pst{sc}_{oc}")
                    pss[(sc, oc)] = ps
                    for jw in range(3):
                        nc.tensor.matmul(
                            ps[:], ctxT[qb][jw][:, sc * 128:(sc + 1) * 128],
                            wo_m[:, jw, osl],
                            start=(jw == 0), stop=False)
                def tail_copy(sc, oc, ps):
                    osl = slice(oc * 512, (oc + 1) * 512)
                    if oc == 0:
                        nc.scalar.activation(ots[sc][:, osl], ps[:], AF.Copy,
                                             bias=0.0, scale=1.0)
                    else:
                        nc.vector.tensor_copy(ots[sc][:, osl], ps[:])

                def tail_dma(sc, osl=slice(0, D)):
                    sg = qb * (SB // 128) + sc
                    eng = nc.scalar if sc % 2 == 0 else nc.sync
                    eng.dma_start(outd[sg * 128:(sg + 1) * 128, osl],
                                  ots[sc][:, osl])

                pendn()
                for sc, oc in groups[:4]:
                    osl = slice(oc * 512, (oc + 1) * 512)
                    ps = pss[(sc, oc)]
                    nc.tensor.matmul(
                        ps[:], ctxT[qb][3][:, sc * 128:(sc + 1) * 128],
                        wo_m[:, 3, osl], start=False, stop=True)
                    tail_copy(sc, oc, ps)
                    if oc == 1:
                        tail_dma(sc)
                for gi, (sc, oc) in enumerate(groups[4:]):
                    osl = slice(oc * 512, (oc + 1) * 512)
                    ps = psum.tile([128, 512], F32, tag=trest[gi],
                                   name=f"pst2_{sc}_{oc}")
                    for jw in range(4):
                        nc.tensor.matmul(
                            ps[:], ctxT[qb][jw][:, sc * 128:(sc + 1) * 128],
                            wo_m[:, jw, osl],
                            start=(jw == 0), stop=(jw == 3))
                    tail_copy(sc, oc, ps)
                    if sc < 3 and oc == 1:
                        tail_dma(sc)
                    elif sc == 3:
                        tail_dma(sc, osl)

            def make_filler(ts):
                us = []
                if ts + 1 < n_st:
                    for m in range(4):
                        us.append(lambda ts=ts, m=m: proj_q_unit(ts + 1, m))
                if ts == 1:
                    for sc in range(4):
                        us.append(lambda sc=sc: outproj_unit(0, sc))
                if ts == 2:
                    for sc in range(2):
                        us.append(lambda sc=sc: outproj_unit(1, sc))
                if ts == 3:
                    for sc in range(2, 4):
                        us.append(lambda sc=sc: outproj_unit(1, sc))
                    for sc in range(4):
                        us.append(lambda sc=sc: outproj_unit(2, sc))
                if ts + 1 < n_st:
                    for m in range(4):
                        us.append(lambda ts=ts, m=m: proj_k_unit(ts + 1, m))
                    for sc in range(4):
                        us.append(lambda ts=ts, sc=sc: proj_v_unit(ts + 1, sc))
                return us

            # ---- attention: software-pipelined scores/exp -> ctx with PE
            # filler between the steps; the normalize of pair j is emitted
            # after pair j+1's first scores ----
            def attn(qb, filler, defer_final_norm=False):
                nt = 4 * (qb + 1)
                n_steps = 4 * nt
                done = [0]
                step = [0]

                # last stage: drain the filler by ~80% so its tail does not
                # collide with the final output projection
                denom = (n_steps + 8) if qb + 1 < 4 else max(1, n_steps - 12)

                def pop():
                    step[0] += 1
                    want = min(len(filler),
                               (len(filler) * step[0]) // denom)
                    while done[0] < want:
                        filler[done[0]]()
                        done[0] += 1

                def normalize(j, c0, c1):
                    with nc.allow_low_precision(reason="bf16 softmax"):
                        r0 = npool.tile([1, QB], F32, tag="r0", name="r0")
                        r1 = npool.tile([1, QB], F32, tag="r1", name="r1")
                        cs0 = npool.tile([DK, QB], BF16, tag="cs0", name="cs0")
                        cs1 = npool.tile([DK, QB], BF16, tag="cs1", name="cs1")
                        # NOTE: reciprocal_approx_fast (custom DVE op)
                        # returns garbage on actual hardware here -- keep the
                        # plain DVE reciprocal.
                        nc.vector.reciprocal(r0[:], c0[DK:DK + 1, :])
                        nc.vector.reciprocal(r1[:], c1[DK:DK + 1, :])
                        if qb + 1 < 4:
                            nc.scalar.activation(cs0[:], c0[0:DK, :], AF.Copy,
                                                 bias=0.0, scale=1.0)
                        else:
                            # last stage is ACT-paced: keep the copy off ACT
                            nc.vector.tensor_copy(cs0[:], c0[0:DK, :])
                        nc.vector.tensor_copy(cs1[:], c1[0:DK, :])
                        rb0 = npool.tile([DK, QB], F32, tag="rb0", name="rb0")
                        rb1 = npool.tile([DK, QB], F32, tag="rb1", name="rb1")
                        nc.gpsimd.partition_broadcast(rb0[:], r0[:])
                        nc.gpsimd.partition_broadcast(rb1[:], r1[:])
                        nc.gpsimd.tensor_tensor(
                            ctxT[qb][j][0:64, :], cs0[:], rb0[:],
                            op=mybir.AluOpType.mult)
                        nc.gpsimd.tensor_tensor(
                            ctxT[qb][j][64:128, :], cs1[:], rb1[:],
                            op=mybir.AluOpType.mult)

                pend = [None]
                for j in range(4):          # head pairs
                    h0, h1 = 2 * j, 2 * j + 1
                    eb = [None] * nt
                    lob = [0] * nt

                    def scores(t, j=j, eb=eb, lob=lob):
                        tks, tkc = t // 4, t % 4
                        ksl = slice(tkc * KT, (tkc + 1) * KT)
                        jj = t - 4 * qb
                        lo = jj * KT if jj > 0 else 0
                        lob[t] = lo
                        s01 = psum.tile([128, 2, QB], F32, tag=f"sc{t % 2}",
                                        name=f"s01_{qb}_{j}_{t}")
                        nc.tensor.matmul(
                            s01[:, 0, lo:], khT[tks][j][0:64, ksl],
                            qhT[qb][j][0:64, lo:], start=True, stop=True)
                        nc.tensor.matmul(
                            s01[:, 1, lo:], khT[tks][j][64:128, ksl],
                            qhT[qb][j][64:128, lo:], start=True, stop=True,
                            tile_position=(64, 0))
                        e01 = epool.tile([128, 2, QB], BF16, tag="e01",
                                         name=f"e01_{qb}_{j}_{t}")
                        nc.scalar.activation(e01[:, :, lo:], s01[:, :, lo:],
                                             AF.Exp, scale=scale)
                        if jj >= 0:
                            nc.vector.tensor_mul(
                                e01[:, :, lo:lo + KT], e01[:, :, lo:lo + KT],
                                masks[:].unsqueeze(1).broadcast_to(
                                    [128, 2, KT]))
                        eb[t] = e01

                    scores(0)
                    if pend[0] is not None:
                        pend[0]()
                        pend[0] = None
                    c0 = psum.tile([DK + 1, QB], F32, tag="c0",
                                   name=f"c0_{qb}_{j}")
                    c1 = psum.tile([DK + 1, QB], F32, tag="c1",
                                   name=f"c1_{qb}_{j}")

                    def ctx(t, c0=c0, c1=c1, h0=h0, h1=h1, eb=eb, lob=lob):
                        lo = lob[t]
                        nc.tensor.matmul(
                            c0[:, lo:], vh[t][:, h0, :], eb[t][:, 0, lo:],
                            start=(t == 0), stop=(t == nt - 1))
                        nc.tensor.matmul(
                            c1[:, lo:], vh[t][:, h1, :], eb[t][:, 1, lo:],
                            start=(t == 0), stop=(t == nt - 1))

                    for t in range(1, nt):
                        scores(t)
                        pop()
                        ctx(t - 1)
                    pop()
                    ctx(nt - 1)
                    pend[0] = (lambda j=j, c0=c0, c1=c1: normalize(j, c0, c1))
                # leftover filler first so its PSUM consumers don't queue
                # behind the final normalize on DVE
                while done[0] < len(filler):
                    filler[done[0]]()
                    done[0] += 1
                if defer_final_norm:
                    return pend[0]
                pend[0]()

            # ---- pipeline ----
            proj_stage0()
            pendn = None
            for ts in range(n_st):
                if ts + 2 < n_st:
                    stage_x_dma(ts + 2)
                if ts + 1 < n_st:
                    attn(ts, make_filler(ts))
                else:
                    pendn = attn(ts, make_filler(ts), defer_final_norm=True)
            outproj_tail(n_st - 1, pendn)

            psum.release()

    nc.compile()
    return nc


def _get_nc(s=S):
    if s not in _CACHE:
        _CACHE[s] = _build(s)
    return _CACHE[s]


def _make_masks(s=S):
    # triangular strip: valid iff local q index >= local k index
    m = np.zeros((KT, KT), np.float32)
    for kk in range(KT):
        m[kk, kk:] = 1.0
    return m.astype(ml_dtypes.bfloat16)


def make_in_maps(q, k, v, Wq, bq, Wk, bk, Wv, bv, Wo, s=S):
    BF = ml_dtypes.bfloat16
    masks = _make_masks(s)
    qT = [np.ascontiguousarray(q[b].T).astype(BF) for b in range(B)]
    kT = [np.ascontiguousarray(k[b].T).astype(BF) for b in range(B)]
    vT = [np.ascontiguousarray(v[b].T).astype(BF) for b in range(B)]
    in_maps = []
    for c in range(N_CORES):
        b, g = c // 2, c % 2
        gsl = slice(g * O, (g + 1) * O)
        in_maps.append({
            "xqT": qT[b],
            "xkT": kT[b],
            "xvT": vT[b],
            "qpk": np.ascontiguousarray(np.concatenate(
                [Wq[gsl, :].T.astype(BF), qT[b][:, 0:SB]], axis=1)),
            "kpk": np.ascontiguousarray(np.concatenate(
                [Wk[gsl, :].T.astype(BF), kT[b][:, 0:SB]], axis=1)),
            "vpk": np.ascontiguousarray(np.concatenate(
                [Wv[gsl, :].T.astype(BF), vT[b][:, 0:SB]], axis=1)),
            "bq": np.ascontiguousarray(bq[gsl]),
            "bk": np.ascontiguousarray(bk[gsl]),
            "bv_bc": np.ascontiguousarray(
                np.broadcast_to(bv[gsl][None, :], (128, O))).astype(BF),
            "woT": np.ascontiguousarray(Wo[:, gsl].T).astype(BF),
            "masks": masks,
            "ones8": np.ones((128, HPC), ml_dtypes.bfloat16),
        })
    return in_maps


def kernel(q, k, v, mask, Wq, bq, Wk, bk, Wv, bv, Wo, bo):
    q = np.asarray(q, np.float32)
    k = np.asarray(k, np.float32)
    v = np.asarray(v, np.float32)
    nc = _get_nc(S)
    in_maps = make_in_maps(q, k, v,
                           np.asarray(Wq, np.float32), np.asarray(bq, np.float32),
                           np.asarray(Wk, np.float32), np.asarray(bk, np.float32),
                           np.asarray(Wv, np.float32), np.asarray(bv, np.float32),
                           np.asarray(Wo, np.float32), S)
    res = run_bass_kernel_spmd(nc, in_maps, list(range(N_CORES)))
    bo = np.asarray(bo, np.float32)
    out = np.empty((B, S, D), np.float32)
    for b in range(B):
        out[b] = (np.asarray(res.results[2 * b]["out"], np.float32)
                  + np.asarray(res.results[2 * b + 1]["out"], np.float32)
                  + bo)
    return out
